# revision 7
# baseline (speedup 1.0000x reference)
"""CTRNN cell (RK4, 6 unfolds) as a Bass/Tile kernel on 8 Trainium2 cores.

Data-parallel: batch (32768) sharded 8 ways; weights replicated; no
cross-core communication.  Per core: 4096 batch rows processed as 8
chunks of 512 batch-columns in the units-on-partitions (transposed)
layout, interleaved in groups of 4.

Schedule (v2, rebuilt from hardware NTFF profiles of the previous
per-ub-block version):
  - chunk-major matmul order: per (chunk, stage) one [128, 2048] mega-PSUM
    tile (4 banks, double buffered) filled by 20 back-to-back matmuls
    (4 identity-injections of xb + 16 R-block matmuls), evacuated by a
    single 2048-wide tanh on ACT immediately after.  Long, gap-free PE
    stretches keep the tensor engine out of its low p-states and off the
    hardware utilization throttle (the old schedule ran every matmul at
    the 1.2 GHz mid p-state; this one averages ~380 ns per 512-col matmul
    vs 440 ns before).
  - gpsimd (Pool) is left idle: its tensor ops run 3-4x slower than DVE
    and contend with DVE for SBUF ports (measured: moving work to Pool
    slowed the remaining DVE stts by up to 50%).
  - engine assignment by measured op rates: plain TENSOR_SCALAR hits the
    DVE fast path (~0.35 ns/elem), TENSOR_TENSOR runs at 2x only when all
    operands are 2-byte, and scalar_tensor_tensor never gets a fast mode
    (~1.1 ns/elem) but is still the cheapest way to touch the f32 h state
    in one instruction.
  - RK4 elementwise per stage: d = s*t - v via per-slice TENSOR_SCALAR
    (even stages on DVE, odd on ACT Identity) + one mega subtract;
    v_next = v1 + c_j*d via fast TS-mul + TT-add against the step's bf16
    copy of h; the RK4 combination d1 + 2*(d2+d3) + d4 is folded into
    d1's tile (one stt + one TT) so the h update is a single stt and the
    step-end critical chain is tanh -> d4 -> E += d4 -> h += dt/6*E.
  - h accumulates in one in-place-updated f32 tile per chunk (bf16 h
    costs ~8e-3 relative error; the f32 tile plus a single per-step bf16
    cast costs one ACT copy).

Input specialization: setup_inputs() for this problem fixes scale == 1
and bias == 0.  kernel() checks the actual values at run time and
dispatches to a build that skips the scale multiplies and the bias add
(identical arithmetic for those inputs); any other inputs take the
general build.  Both builds measure ~4e-3 relative error vs the f32 jax
reference.

Measured on hardware (NTFF profile, symmetric across the 8 cores):
~1.03 ms NEFF execution per core, down from 2.17 ms for the previous
schedule; PE idle < 15% with matmuls ~80% of the span.
"""

from contextlib import ExitStack

import numpy as np

_B, _DIN, _UNITS = 32768, 256, 512
_NCORES = 8
_BLOCAL = _B // _NCORES      # 4096
_CHUNK = 512                 # batch columns per chunk
_NCHUNKS = _BLOCAL // _CHUNK # 8
_NSTEPS = 6
_DT = 1.0 / _NSTEPS

_cached = {}


def _build_program(n_chunks=_NCHUNKS, n_steps=_NSTEPS, group=4,
                   specialize=False, idmm=True, cast_eng="act"):
    import concourse.bass as bass
    import concourse.tile as tile
    from concourse import bacc, mybir
    from concourse.masks import make_identity

    f32 = mybir.dt.float32
    bf16 = mybir.dt.bfloat16
    Alu = mybir.AluOpType
    Act = mybir.ActivationFunctionType

    UB = _UNITS // 128    # 4 unit blocks
    DB = _DIN // 128      # 2 d_in blocks
    BB = _CHUNK // 128    # 4 batch blocks per chunk
    W = UB * _CHUNK       # 2048

    b_rows = n_chunks * _CHUNK
    assert n_chunks % group == 0

    nc = bacc.Bacc("TRN2", target_bir_lowering=False, debug=False)

    x_d = nc.dram_tensor("x", [b_rows, _DIN], f32, kind="ExternalInput")
    h_d = nc.dram_tensor("h0", [b_rows, _UNITS], f32, kind="ExternalInput")
    K_d = nc.dram_tensor("Kw", [_DIN, _UNITS], f32, kind="ExternalInput")
    R_d = nc.dram_tensor("Rw", [_UNITS, _UNITS], f32, kind="ExternalInput")
    b_d = nc.dram_tensor("bv", [_UNITS], f32, kind="ExternalInput")
    s_d = nc.dram_tensor("sv", [_UNITS], f32, kind="ExternalInput")
    o_d = nc.dram_tensor("out", [b_rows, _UNITS], f32, kind="ExternalOutput")

    with tile.TileContext(nc) as tc, ExitStack() as ctx:
        wpool = ctx.enter_context(tc.tile_pool(name="w", bufs=1))
        stgpool = ctx.enter_context(tc.tile_pool(name="stg", bufs=2))
        iopool = ctx.enter_context(tc.tile_pool(name="io", bufs=2))
        hpool = ctx.enter_context(tc.tile_pool(name="h", bufs=group))
        xbpool = ctx.enter_context(tc.tile_pool(name="xb", bufs=group))
        vpool = ctx.enter_context(tc.tile_pool(name="vn", bufs=group + 2))
        v1pool = ctx.enter_context(tc.tile_pool(name="v1", bufs=group + 1))
        tpool = ctx.enter_context(tc.tile_pool(name="t", bufs=2))
        dpool = ctx.enter_context(tc.tile_pool(name="d", bufs=3))
        apool = ctx.enter_context(tc.tile_pool(name="acc", bufs=group))
        opool = ctx.enter_context(tc.tile_pool(name="o", bufs=1))
        pspool = ctx.enter_context(tc.tile_pool(name="ps", bufs=2, space="PSUM"))

        # ---- weights / constants ----
        R_sb = []
        for kb in range(UB):
            stg = stgpool.tile([128, _UNITS], f32, tag="stg")
            nc.sync.dma_start(out=stg[:], in_=R_d[kb * 128:(kb + 1) * 128, :])
            t_ = wpool.tile([128, _UNITS], bf16, tag=f"R{kb}")
            nc.vector.tensor_copy(t_[:], stg[:])
            R_sb.append(t_)
        K_sb = []
        for db in range(DB):
            stg = stgpool.tile([128, _UNITS], f32, tag="stg")
            nc.sync.dma_start(out=stg[:], in_=K_d[db * 128:(db + 1) * 128, :])
            t_ = wpool.tile([128, _UNITS], bf16, tag=f"K{db}")
            nc.vector.tensor_copy(t_[:], stg[:])
            K_sb.append(t_)
        bias_sb = wpool.tile([128, UB], f32, tag="bias")
        nc.sync.dma_start(out=bias_sb[:], in_=b_d[:].rearrange("(j p) -> p j", p=128))
        scale_sb = wpool.tile([128, UB], f32, tag="scale")
        nc.sync.dma_start(out=scale_sb[:], in_=s_d[:].rearrange("(j p) -> p j", p=128))
        ident = wpool.tile([128, 128], f32, tag="ident")
        make_identity(nc, ident[:])
        identW = wpool.tile([128, 128], bf16, tag="identW")
        nc.vector.tensor_copy(identW[:], ident[:])

        def mm(ps_ap, lhsT_ap, rhs_ap, start, stop):
            nc.tensor.matmul(ps_ap, lhsT_ap, rhs_ap, start=start, stop=stop)

        def emit_loads(chunks):
            loads = {}
            for c in chunks:
                r0 = c * _CHUNK
                xn, hn = [], []
                for bb in range(BB):
                    t_ = iopool.tile([128, _DIN], f32, tag=f"xn{bb}")
                    nc.sync.dma_start(
                        out=t_[:], in_=x_d[r0 + bb * 128:r0 + (bb + 1) * 128, :]
                    )
                    xn.append(t_)
                for bb in range(BB):
                    t_ = iopool.tile([128, _UNITS], f32, tag=f"hn{bb}")
                    nc.sync.dma_start(
                        out=t_[:], in_=h_d[r0 + bb * 128:r0 + (bb + 1) * 128, :]
                    )
                    hn.append(t_)
                loads[c] = (xn, hn)
            return loads

        groups = [list(range(g0, g0 + group))
                  for g0 in range(0, n_chunks, group)]
        next_loads = emit_loads(groups[0])

        for gi, chunks in enumerate(groups):
            st = {c: {} for c in chunks}
            loads = next_loads

            # ---- per-chunk init: transpose, xb ----
            for c in chunks:
                r0 = c * _CHUNK
                xn, hn = loads[c]

                xT = iopool.tile([128, DB * _CHUNK], bf16, tag="xT")
                ps = pspool.tile([128, W], f32, tag="ps")
                for db in range(DB):
                    for bb in range(BB):
                        nc.tensor.transpose(
                            ps[:, db * _CHUNK + bb * 128:db * _CHUNK + (bb + 1) * 128],
                            xn[bb][:, db * 128:(db + 1) * 128],
                            ident[:],
                        )
                nc.vector.tensor_copy(xT[:], ps[:, :DB * _CHUNK])

                hT = hpool.tile([128, W], f32, tag="hT")
                ps = pspool.tile([128, W], f32, tag="ps")
                for ub in range(UB):
                    for bb in range(BB):
                        nc.tensor.transpose(
                            ps[:, ub * _CHUNK + bb * 128:ub * _CHUNK + (bb + 1) * 128],
                            hn[bb][:, ub * 128:(ub + 1) * 128],
                            ident[:],
                        )
                nc.scalar.copy(hT[:], ps[:])
                st[c]["hT"] = hT

                # xbT = (x @ K).T (+ bias unless specialized away)
                xbT = xbpool.tile([128, W], bf16, tag="xbT")
                ps = pspool.tile([128, W], f32, tag="ps")
                for ub in range(UB):
                    for db in range(DB):
                        mm(
                            ps[:, ub * _CHUNK:(ub + 1) * _CHUNK],
                            K_sb[db][:, ub * 128:(ub + 1) * 128],
                            xT[:, db * _CHUNK:(db + 1) * _CHUNK],
                            start=(db == 0),
                            stop=(db == DB - 1),
                        )
                if specialize:
                    nc.vector.tensor_copy(xbT[:], ps[:])
                else:
                    for ub in range(UB):
                        nc.vector.tensor_scalar_add(
                            xbT[:, ub * _CHUNK:(ub + 1) * _CHUNK],
                            ps[:, ub * _CHUNK:(ub + 1) * _CHUNK],
                            bias_sb[:, ub:ub + 1],
                        )
                st[c]["xbT"] = xbT

                v1 = v1pool.tile([128, W], bf16, tag="v1")
                (nc.scalar.copy if cast_eng == "act" else nc.vector.tensor_copy)(
                    v1[:], hT[:]
                )
                st[c]["v1"] = v1
                st[c]["vcur"] = v1

            # ---- RK4 unfold steps, chunk-interleaved per stage ----
            for s_i in range(n_steps):
                for j in range(4):
                    cj = (_DT / 2.0) if j < 2 else _DT
                    # v1 casts are emitted one chunk late so they never
                    # head-of-line-block the next chunk's tanh on ACT (the
                    # cast waits on the DVE h-update; tanh frees PSUM)
                    pending_cast = []

                    def flush_cast():
                        cp = pending_cast.pop(0)
                        v1 = v1pool.tile([128, W], bf16, tag="v1")
                        (nc.scalar.copy if cast_eng == "act"
                         else nc.vector.tensor_copy)(v1[:], st[cp]["hT"][:])
                        st[cp]["v1"] = v1
                        st[cp]["vcur"] = v1

                    for c in chunks:
                        hT = st[c]["hT"]
                        vcur = st[c]["vcur"]
                        xbT = st[c]["xbT"]

                        if j == 3:
                            # E = d1 + 2*(d2 + d3), folded into d1's tile
                            # before this chunk's matmuls; the step-end
                            # critical chain is then tanh -> d4 -> E += d4
                            # -> one h stt
                            nc.vector.scalar_tensor_tensor(
                                st[c]["accA"][:], st[c]["accB"][:], 2.0,
                                st[c]["accA"][:], Alu.mult, Alu.add,
                            )

                        ps = pspool.tile([128, W], f32, tag="ps")
                        if not idmm:
                            nc.scalar.copy(ps[:], xbT[:])
                        for ub in range(UB):
                            sl = slice(ub * _CHUNK, (ub + 1) * _CHUNK)
                            if idmm:
                                mm(ps[:, sl], identW[:], xbT[:, sl],
                                   start=True, stop=False)
                            for kb in range(UB):
                                nc.tensor.matmul(
                                    ps[:, sl],
                                    R_sb[kb][:, ub * 128:(ub + 1) * 128],
                                    vcur[:, kb * _CHUNK:(kb + 1) * _CHUNK],
                                    start=False,
                                    stop=(kb == UB - 1),
                                    skip_group_check=(not idmm),
                                )

                        t_t = tpool.tile([128, W], bf16, tag="t")
                        nc.scalar.activation(t_t[:], ps[:], Act.Tanh)

                        if j == 0:
                            dtile = apool.tile([128, W], bf16, tag="accA")
                            st[c]["accA"] = dtile
                        elif j == 1:
                            dtile = apool.tile([128, W], bf16, tag="accB")
                            st[c]["accB"] = dtile
                        else:
                            dtile = dpool.tile([128, W], bf16, tag="d")

                        # d = s*t - vcur (scale pass vanishes when s == 1)
                        if specialize:
                            src = t_t
                        else:
                            src = tpool.tile([128, W], bf16, tag="ts")
                            for ub in range(UB):
                                sl = slice(ub * _CHUNK, (ub + 1) * _CHUNK)
                                if j % 2 == 0:
                                    nc.vector.tensor_scalar_mul(
                                        src[:, sl], t_t[:, sl],
                                        scale_sb[:, ub:ub + 1]
                                    )
                                else:
                                    nc.scalar.activation(
                                        src[:, sl], t_t[:, sl], Act.Identity,
                                        scale=scale_sb[:, ub:ub + 1],
                                    )
                        nc.vector.tensor_sub(dtile[:], src[:], vcur[:])

                        if j < 3:
                            # vn = v1 + cj*d via fast TS-mul + bf16 TT-add
                            dv = tpool.tile([128, W], bf16, tag="dv")
                            nc.vector.tensor_scalar_mul(dv[:], dtile[:], cj)
                            vn = vpool.tile([128, W], bf16, tag="vn")
                            nc.vector.tensor_add(vn[:], dv[:], st[c]["v1"][:])
                            st[c]["vcur"] = vn

                        if j == 2:
                            nc.vector.tensor_add(
                                st[c]["accB"][:], st[c]["accB"][:], dtile[:]
                            )
                        elif j == 3:
                            E = st[c]["accA"]
                            nc.vector.tensor_add(E[:], E[:], dtile[:])
                            nc.vector.scalar_tensor_tensor(
                                st[c]["hT"][:], E[:], _DT / 6.0,
                                st[c]["hT"][:], Alu.mult, Alu.add,
                            )
                            if s_i < n_steps - 1:
                                pending_cast.append(c)
                                if len(pending_cast) > 1:
                                    flush_cast()
                    while pending_cast:
                        flush_cast()

            # prefetch the next group's inputs before this group's output
            # phase so its transposes never wait on cold DMAs
            if gi + 1 < len(groups):
                next_loads = emit_loads(groups[gi + 1])

            # ---- output: transpose back and store ----
            for c in chunks:
                r0 = c * _CHUNK
                hT = st[c]["hT"]
                ps = pspool.tile([128, W], f32, tag="ps")
                for bb in range(BB):
                    for ub in range(UB):
                        nc.tensor.transpose(
                            ps[:, bb * _CHUNK + ub * 128:bb * _CHUNK + (ub + 1) * 128],
                            hT[:, ub * _CHUNK + bb * 128:ub * _CHUNK + (bb + 1) * 128],
                            ident[:],
                        )
                o_sb = opool.tile([128, W], f32, tag="o")
                nc.scalar.copy(o_sb[:], ps[:])
                for bb in range(BB):
                    nc.sync.dma_start(
                        out=o_d[r0 + bb * 128:r0 + (bb + 1) * 128, :],
                        in_=o_sb[:, bb * _CHUNK:(bb + 1) * _CHUNK],
                    )

    nc.compile()
    return nc


def _get_program(specialize):
    key = ("spec" if specialize else "gen")
    if key not in _cached:
        _cached[key] = _build_program(specialize=specialize)
    return _cached[key]


def _make_in_maps(inputs, hidden_state, kern, recurrent_kernel, bias, scale):
    def f(a):
        return np.ascontiguousarray(np.asarray(a), dtype=np.float32)

    x = f(inputs)
    h = f(hidden_state)
    shared = {
        "Kw": f(kern),
        "Rw": f(recurrent_kernel),
        "bv": f(bias),
        "sv": f(scale),
    }
    maps = []
    for c in range(_NCORES):
        sl = slice(c * _BLOCAL, (c + 1) * _BLOCAL)
        maps.append({"x": x[sl], "h0": h[sl], **shared})
    return maps


def _can_specialize(bias, scale):
    b = np.asarray(bias)
    s = np.asarray(scale)
    return bool(np.all(b == 0.0)) and bool(np.all(s == 1.0))


def _run(in_maps, specialize, trace=False):
    from concourse.bass_utils import run_bass_kernel_spmd

    nc = _get_program(specialize)
    res = run_bass_kernel_spmd(nc, in_maps, list(range(_NCORES)), trace=trace)
    out = np.concatenate(
        [res.results[i]["out"] for i in range(_NCORES)], axis=0
    ).astype(np.float32)
    return out, res


def kernel(inputs, hidden_state, kernel, recurrent_kernel, bias, scale):
    in_maps = _make_in_maps(inputs, hidden_state, kernel, recurrent_kernel,
                            bias, scale)
    # scale == 1 / bias == 0 (this problem's setup) takes the build that
    # skips those ops entirely; anything else takes the general build
    out, _ = _run(in_maps, specialize=_can_specialize(bias, scale))
    return out


# revision 8
# speedup vs baseline: 1.0115x; 1.0115x over previous
"""CTRNN cell (RK4, 6 unfolds) as a Bass/Tile kernel on 8 Trainium2 cores.

Data-parallel: batch (32768) sharded 8 ways; weights replicated; no
cross-core communication.  Per core: 4096 batch rows processed as 8
chunks of 512 batch-columns in the units-on-partitions (transposed)
layout, interleaved in groups of 4.

Schedule (v2, rebuilt from hardware NTFF profiles of the previous
per-ub-block version):
  - chunk-major matmul order: per (chunk, stage) one [128, 2048] mega-PSUM
    tile (4 banks, double buffered) filled by 20 back-to-back matmuls
    (4 identity-injections of xb + 16 R-block matmuls), evacuated by a
    single 2048-wide tanh on ACT immediately after.  Long, gap-free PE
    stretches keep the tensor engine out of its low p-states and off the
    hardware utilization throttle (the old schedule ran every matmul at
    the 1.2 GHz mid p-state; this one averages ~380 ns per 512-col matmul
    vs 440 ns before).
  - gpsimd (Pool) is left idle: its tensor ops run 3-4x slower than DVE
    and contend with DVE for SBUF ports (measured: moving work to Pool
    slowed the remaining DVE stts by up to 50%).
  - engine assignment by measured op rates: plain TENSOR_SCALAR hits the
    DVE fast path (~0.35 ns/elem), TENSOR_TENSOR runs at 2x only when all
    operands are 2-byte, and scalar_tensor_tensor never gets a fast mode
    (~1.1 ns/elem) but is still the cheapest way to touch the f32 h state
    in one instruction.
  - RK4 elementwise per stage: d = s*t - v via per-slice TENSOR_SCALAR
    (even stages on DVE, odd on ACT Identity) + one mega subtract;
    v_next = v1 + c_j*d via fast TS-mul + TT-add against the step's bf16
    copy of h; the RK4 combination d1 + 2*(d2+d3) + d4 is folded into
    d1's tile (one stt + one TT) so the h update is a single stt and the
    step-end critical chain is tanh -> d4 -> E += d4 -> h += dt/6*E.
  - h accumulates in one in-place-updated f32 tile per chunk (bf16 h
    costs ~8e-3 relative error; the f32 tile plus a single per-step bf16
    cast costs one ACT copy).

Input specialization: setup_inputs() for this problem fixes scale == 1
and bias == 0.  kernel() checks the actual values at run time and
dispatches to a build that skips the scale multiplies and the bias add
(identical arithmetic for those inputs); any other inputs take the
general build.  Both builds measure ~4e-3 relative error vs the f32 jax
reference.

Measured on hardware (NTFF profile, symmetric across the 8 cores):
~1.03 ms NEFF execution per core, down from 2.17 ms for the previous
schedule; PE idle < 15% with matmuls ~80% of the span.
"""

from contextlib import ExitStack

import numpy as np

_B, _DIN, _UNITS = 32768, 256, 512
_NCORES = 8
_BLOCAL = _B // _NCORES      # 4096
_CHUNK = 512                 # batch columns per chunk
_NCHUNKS = _BLOCAL // _CHUNK # 8
_NSTEPS = 6
_DT = 1.0 / _NSTEPS

_cached = {}


def _build_program(n_chunks=_NCHUNKS, n_steps=_NSTEPS, group=4,
                   specialize=False, idmm=True, cast_eng="act"):
    import concourse.bass as bass
    import concourse.tile as tile
    from concourse import bacc, mybir
    from concourse.masks import make_identity

    f32 = mybir.dt.float32
    bf16 = mybir.dt.bfloat16
    Alu = mybir.AluOpType
    Act = mybir.ActivationFunctionType

    UB = _UNITS // 128    # 4 unit blocks
    DB = _DIN // 128      # 2 d_in blocks
    BB = _CHUNK // 128    # 4 batch blocks per chunk
    W = UB * _CHUNK       # 2048

    b_rows = n_chunks * _CHUNK
    assert n_chunks % group == 0

    nc = bacc.Bacc("TRN2", target_bir_lowering=False, debug=False)

    x_d = nc.dram_tensor("x", [b_rows, _DIN], f32, kind="ExternalInput")
    h_d = nc.dram_tensor("h0", [b_rows, _UNITS], f32, kind="ExternalInput")
    K_d = nc.dram_tensor("Kw", [_DIN, _UNITS], f32, kind="ExternalInput")
    R_d = nc.dram_tensor("Rw", [_UNITS, _UNITS], f32, kind="ExternalInput")
    b_d = nc.dram_tensor("bv", [_UNITS], f32, kind="ExternalInput")
    s_d = nc.dram_tensor("sv", [_UNITS], f32, kind="ExternalInput")
    o_d = nc.dram_tensor("out", [b_rows, _UNITS], f32, kind="ExternalOutput")

    with tile.TileContext(nc) as tc, ExitStack() as ctx:
        wpool = ctx.enter_context(tc.tile_pool(name="w", bufs=1))
        stgpool = ctx.enter_context(tc.tile_pool(name="stg", bufs=2))
        iopool = ctx.enter_context(tc.tile_pool(name="io", bufs=2))
        hpool = ctx.enter_context(tc.tile_pool(name="h", bufs=group))
        xbpool = ctx.enter_context(tc.tile_pool(name="xb", bufs=group))
        vpool = ctx.enter_context(tc.tile_pool(name="vn", bufs=group + 2))
        v1pool = ctx.enter_context(tc.tile_pool(name="v1", bufs=group + 1))
        tpool = ctx.enter_context(tc.tile_pool(name="t", bufs=2))
        dpool = ctx.enter_context(tc.tile_pool(name="d", bufs=3))
        apool = ctx.enter_context(tc.tile_pool(name="acc", bufs=group))
        opool = ctx.enter_context(tc.tile_pool(name="o", bufs=2))
        pspool = ctx.enter_context(tc.tile_pool(name="ps", bufs=2, space="PSUM"))

        # ---- weights / constants ----
        R_sb = []
        for kb in range(UB):
            stg = stgpool.tile([128, _UNITS], f32, tag="stg")
            nc.sync.dma_start(out=stg[:], in_=R_d[kb * 128:(kb + 1) * 128, :])
            t_ = wpool.tile([128, _UNITS], bf16, tag=f"R{kb}")
            nc.vector.tensor_copy(t_[:], stg[:])
            R_sb.append(t_)
        K_sb = []
        for db in range(DB):
            stg = stgpool.tile([128, _UNITS], f32, tag="stg")
            nc.sync.dma_start(out=stg[:], in_=K_d[db * 128:(db + 1) * 128, :])
            t_ = wpool.tile([128, _UNITS], bf16, tag=f"K{db}")
            nc.vector.tensor_copy(t_[:], stg[:])
            K_sb.append(t_)
        bias_sb = wpool.tile([128, UB], f32, tag="bias")
        nc.sync.dma_start(out=bias_sb[:], in_=b_d[:].rearrange("(j p) -> p j", p=128))
        scale_sb = wpool.tile([128, UB], f32, tag="scale")
        nc.sync.dma_start(out=scale_sb[:], in_=s_d[:].rearrange("(j p) -> p j", p=128))
        ident = wpool.tile([128, 128], f32, tag="ident")
        make_identity(nc, ident[:])
        identW = wpool.tile([128, 128], bf16, tag="identW")
        nc.vector.tensor_copy(identW[:], ident[:])

        def mm(ps_ap, lhsT_ap, rhs_ap, start, stop):
            nc.tensor.matmul(ps_ap, lhsT_ap, rhs_ap, start=start, stop=stop)

        def emit_loads(chunks):
            loads = {}
            for c in chunks:
                r0 = c * _CHUNK
                xn, hn = [], []
                for bb in range(BB):
                    t_ = iopool.tile([128, _DIN], f32, tag=f"xn{bb}")
                    nc.sync.dma_start(
                        out=t_[:], in_=x_d[r0 + bb * 128:r0 + (bb + 1) * 128, :]
                    )
                    xn.append(t_)
                for bb in range(BB):
                    t_ = iopool.tile([128, _UNITS], f32, tag=f"hn{bb}")
                    nc.sync.dma_start(
                        out=t_[:], in_=h_d[r0 + bb * 128:r0 + (bb + 1) * 128, :]
                    )
                    hn.append(t_)
                loads[c] = (xn, hn)
            return loads

        groups = [list(range(g0, g0 + group))
                  for g0 in range(0, n_chunks, group)]
        next_loads = emit_loads(groups[0])

        for gi, chunks in enumerate(groups):
            st = {c: {} for c in chunks}
            loads = next_loads

            # ---- per-chunk init: transpose, xb ----
            for c in chunks:
                r0 = c * _CHUNK
                xn, hn = loads[c]

                xT = iopool.tile([128, DB * _CHUNK], bf16, tag="xT")
                ps = pspool.tile([128, W], f32, tag="ps")
                for db in range(DB):
                    for bb in range(BB):
                        nc.tensor.transpose(
                            ps[:, db * _CHUNK + bb * 128:db * _CHUNK + (bb + 1) * 128],
                            xn[bb][:, db * 128:(db + 1) * 128],
                            ident[:],
                        )
                nc.vector.tensor_copy(xT[:], ps[:, :DB * _CHUNK])

                hT = hpool.tile([128, W], f32, tag="hT")
                ps = pspool.tile([128, W], f32, tag="ps")
                for ub in range(UB):
                    for bb in range(BB):
                        nc.tensor.transpose(
                            ps[:, ub * _CHUNK + bb * 128:ub * _CHUNK + (bb + 1) * 128],
                            hn[bb][:, ub * 128:(ub + 1) * 128],
                            ident[:],
                        )
                nc.scalar.copy(hT[:], ps[:])
                st[c]["hT"] = hT

                # xbT = (x @ K).T (+ bias unless specialized away)
                xbT = xbpool.tile([128, W], bf16, tag="xbT")
                ps = pspool.tile([128, W], f32, tag="ps")
                for ub in range(UB):
                    for db in range(DB):
                        mm(
                            ps[:, ub * _CHUNK:(ub + 1) * _CHUNK],
                            K_sb[db][:, ub * 128:(ub + 1) * 128],
                            xT[:, db * _CHUNK:(db + 1) * _CHUNK],
                            start=(db == 0),
                            stop=(db == DB - 1),
                        )
                if specialize:
                    nc.vector.tensor_copy(xbT[:], ps[:])
                else:
                    for ub in range(UB):
                        nc.vector.tensor_scalar_add(
                            xbT[:, ub * _CHUNK:(ub + 1) * _CHUNK],
                            ps[:, ub * _CHUNK:(ub + 1) * _CHUNK],
                            bias_sb[:, ub:ub + 1],
                        )
                st[c]["xbT"] = xbT

                v1 = v1pool.tile([128, W], bf16, tag="v1")
                (nc.scalar.copy if cast_eng == "act" else nc.vector.tensor_copy)(
                    v1[:], hT[:]
                )
                st[c]["v1"] = v1
                st[c]["vcur"] = v1

            # ---- RK4 unfold steps, chunk-interleaved per stage ----
            for s_i in range(n_steps):
                for j in range(4):
                    cj = (_DT / 2.0) if j < 2 else _DT
                    # v1 casts are emitted one chunk late so they never
                    # head-of-line-block the next chunk's tanh on ACT (the
                    # cast waits on the DVE h-update; tanh frees PSUM)
                    pending_cast = []

                    def flush_cast():
                        cp = pending_cast.pop(0)
                        v1 = v1pool.tile([128, W], bf16, tag="v1")
                        (nc.scalar.copy if cast_eng == "act"
                         else nc.vector.tensor_copy)(v1[:], st[cp]["hT"][:])
                        st[cp]["v1"] = v1
                        st[cp]["vcur"] = v1

                    for c in chunks:
                        hT = st[c]["hT"]
                        vcur = st[c]["vcur"]
                        xbT = st[c]["xbT"]

                        if j == 3:
                            # E = d1 + 2*(d2 + d3), folded into d1's tile
                            # before this chunk's matmuls; the step-end
                            # critical chain is then tanh -> d4 -> E += d4
                            # -> one h stt
                            nc.vector.scalar_tensor_tensor(
                                st[c]["accA"][:], st[c]["accB"][:], 2.0,
                                st[c]["accA"][:], Alu.mult, Alu.add,
                            )

                        ps = pspool.tile([128, W], f32, tag="ps")
                        if not idmm:
                            nc.scalar.copy(ps[:], xbT[:])
                        for ub in range(UB):
                            sl = slice(ub * _CHUNK, (ub + 1) * _CHUNK)
                            if idmm:
                                mm(ps[:, sl], identW[:], xbT[:, sl],
                                   start=True, stop=False)
                            for kb in range(UB):
                                nc.tensor.matmul(
                                    ps[:, sl],
                                    R_sb[kb][:, ub * 128:(ub + 1) * 128],
                                    vcur[:, kb * _CHUNK:(kb + 1) * _CHUNK],
                                    start=False,
                                    stop=(kb == UB - 1),
                                    skip_group_check=(not idmm),
                                )

                        t_t = tpool.tile([128, W], bf16, tag="t")
                        nc.scalar.activation(t_t[:], ps[:], Act.Tanh)

                        if j == 0:
                            dtile = apool.tile([128, W], bf16, tag="accA")
                            st[c]["accA"] = dtile
                        elif j == 1:
                            dtile = apool.tile([128, W], bf16, tag="accB")
                            st[c]["accB"] = dtile
                        else:
                            dtile = dpool.tile([128, W], bf16, tag="d")

                        # d = s*t - vcur (scale pass vanishes when s == 1)
                        if specialize:
                            src = t_t
                        else:
                            src = tpool.tile([128, W], bf16, tag="ts")
                            for ub in range(UB):
                                sl = slice(ub * _CHUNK, (ub + 1) * _CHUNK)
                                if j % 2 == 0:
                                    nc.vector.tensor_scalar_mul(
                                        src[:, sl], t_t[:, sl],
                                        scale_sb[:, ub:ub + 1]
                                    )
                                else:
                                    nc.scalar.activation(
                                        src[:, sl], t_t[:, sl], Act.Identity,
                                        scale=scale_sb[:, ub:ub + 1],
                                    )
                        nc.vector.tensor_sub(dtile[:], src[:], vcur[:])

                        if j < 3:
                            # vn = v1 + cj*d via fast TS-mul + bf16 TT-add
                            dv = tpool.tile([128, W], bf16, tag="dv")
                            nc.vector.tensor_scalar_mul(dv[:], dtile[:], cj)
                            vn = vpool.tile([128, W], bf16, tag="vn")
                            nc.vector.tensor_add(vn[:], dv[:], st[c]["v1"][:])
                            st[c]["vcur"] = vn

                        if j == 2:
                            nc.vector.tensor_add(
                                st[c]["accB"][:], st[c]["accB"][:], dtile[:]
                            )
                        elif j == 3:
                            E = st[c]["accA"]
                            nc.vector.tensor_add(E[:], E[:], dtile[:])
                            nc.vector.scalar_tensor_tensor(
                                st[c]["hT"][:], E[:], _DT / 6.0,
                                st[c]["hT"][:], Alu.mult, Alu.add,
                            )
                            if s_i < n_steps - 1:
                                pending_cast.append(c)
                                if len(pending_cast) > 1:
                                    flush_cast()
                    while pending_cast:
                        flush_cast()

            # prefetch the next group's inputs before this group's output
            # phase so its transposes never wait on cold DMAs
            if gi + 1 < len(groups):
                next_loads = emit_loads(groups[gi + 1])

            # ---- output: transpose back and store ----
            for c in chunks:
                r0 = c * _CHUNK
                hT = st[c]["hT"]
                ps = pspool.tile([128, W], f32, tag="ps")
                for bb in range(BB):
                    for ub in range(UB):
                        nc.tensor.transpose(
                            ps[:, bb * _CHUNK + ub * 128:bb * _CHUNK + (ub + 1) * 128],
                            hT[:, ub * _CHUNK + bb * 128:ub * _CHUNK + (bb + 1) * 128],
                            ident[:],
                        )
                o_sb = opool.tile([128, W], f32, tag="o")
                nc.scalar.copy(o_sb[:], ps[:])
                for bb in range(BB):
                    nc.sync.dma_start(
                        out=o_d[r0 + bb * 128:r0 + (bb + 1) * 128, :],
                        in_=o_sb[:, bb * _CHUNK:(bb + 1) * _CHUNK],
                    )

    nc.compile()
    return nc


def _get_program(specialize):
    key = ("spec" if specialize else "gen")
    if key not in _cached:
        _cached[key] = _build_program(specialize=specialize)
    return _cached[key]


def _make_in_maps(inputs, hidden_state, kern, recurrent_kernel, bias, scale):
    def f(a):
        return np.ascontiguousarray(np.asarray(a), dtype=np.float32)

    x = f(inputs)
    h = f(hidden_state)
    shared = {
        "Kw": f(kern),
        "Rw": f(recurrent_kernel),
        "bv": f(bias),
        "sv": f(scale),
    }
    maps = []
    for c in range(_NCORES):
        sl = slice(c * _BLOCAL, (c + 1) * _BLOCAL)
        maps.append({"x": x[sl], "h0": h[sl], **shared})
    return maps


def _can_specialize(bias, scale):
    b = np.asarray(bias)
    s = np.asarray(scale)
    return bool(np.all(b == 0.0)) and bool(np.all(s == 1.0))


def _run(in_maps, specialize, trace=False):
    from concourse.bass_utils import run_bass_kernel_spmd

    nc = _get_program(specialize)
    res = run_bass_kernel_spmd(nc, in_maps, list(range(_NCORES)), trace=trace)
    out = np.concatenate(
        [res.results[i]["out"] for i in range(_NCORES)], axis=0
    ).astype(np.float32)
    return out, res


def kernel(inputs, hidden_state, kernel, recurrent_kernel, bias, scale):
    in_maps = _make_in_maps(inputs, hidden_state, kernel, recurrent_kernel,
                            bias, scale)
    # scale == 1 / bias == 0 (this problem's setup) takes the build that
    # skips those ops entirely; anything else takes the general build
    out, _ = _run(in_maps, specialize=_can_specialize(bias, scale))
    return out


# revision 20
# speedup vs baseline: 1.0125x; 1.0010x over previous
"""CTRNN cell (RK4, 6 unfolds) as a Bass/Tile kernel on 8 Trainium2 cores.

Data-parallel: batch (32768) sharded 8 ways; weights replicated; no
cross-core communication.  Per core: 4096 batch rows processed as 8
chunks of 512 batch-columns in the units-on-partitions (transposed)
layout, interleaved in groups of 4.

Schedule (v2, rebuilt from hardware NTFF profiles of the previous
per-ub-block version):
  - chunk-major matmul order: per (chunk, stage) one [128, 2048] mega-PSUM
    tile (4 banks, double buffered) filled by 20 back-to-back matmuls
    (4 identity-injections of xb + 16 R-block matmuls), evacuated by a
    single 2048-wide tanh on ACT immediately after.  Long, gap-free PE
    stretches keep the tensor engine out of its low p-states and off the
    hardware utilization throttle (the old schedule ran every matmul at
    the 1.2 GHz mid p-state; this one averages ~380 ns per 512-col matmul
    vs 440 ns before).
  - gpsimd (Pool) is left idle: its tensor ops run 3-4x slower than DVE
    and contend with DVE for SBUF ports (measured: moving work to Pool
    slowed the remaining DVE stts by up to 50%).
  - engine assignment by measured op rates: plain TENSOR_SCALAR hits the
    DVE fast path (~0.35 ns/elem), TENSOR_TENSOR runs at 2x only when all
    operands are 2-byte, and scalar_tensor_tensor never gets a fast mode
    (~1.1 ns/elem) but is still the cheapest way to touch the f32 h state
    in one instruction.
  - RK4 elementwise per stage: d = s*t - v via per-slice TENSOR_SCALAR
    (even stages on DVE, odd on ACT Identity) + one mega subtract;
    v_next = v1 + c_j*d via fast TS-mul + TT-add against the step's bf16
    copy of h; the RK4 combination d1 + 2*(d2+d3) + d4 is folded into
    d1's tile (one stt + one TT) so the h update is a single stt and the
    step-end critical chain is tanh -> d4 -> E += d4 -> h += dt/6*E.
  - h accumulates in one in-place-updated f32 tile per chunk (bf16 h
    costs ~8e-3 relative error; the f32 tile plus a single per-step bf16
    cast costs one ACT copy).

Input specialization: setup_inputs() for this problem fixes scale == 1
and bias == 0.  kernel() checks the actual values at run time and
dispatches to a build that skips the scale multiplies and the bias add
(identical arithmetic for those inputs); any other inputs take the
general build.  Both builds measure ~4e-3 relative error vs the f32 jax
reference.

Measured on hardware (NTFF profile, symmetric across the 8 cores):
~1.03 ms NEFF execution per core, down from 2.17 ms for the previous
schedule; PE idle < 15% with matmuls ~80% of the span.
"""

from contextlib import ExitStack

import numpy as np

_B, _DIN, _UNITS = 32768, 256, 512
_NCORES = 8
_BLOCAL = _B // _NCORES      # 4096
_CHUNK = 512                 # batch columns per chunk
_NCHUNKS = _BLOCAL // _CHUNK # 8
_NSTEPS = 6
_DT = 1.0 / _NSTEPS

_cached = {}


def _build_program(n_chunks=_NCHUNKS, n_steps=_NSTEPS, group=4,
                   specialize=False, idmm=True, cast_eng="act"):
    import concourse.bass as bass
    import concourse.tile as tile
    from concourse import bacc, mybir
    from concourse.masks import make_identity

    f32 = mybir.dt.float32
    bf16 = mybir.dt.bfloat16
    Alu = mybir.AluOpType
    Act = mybir.ActivationFunctionType

    UB = _UNITS // 128    # 4 unit blocks
    DB = _DIN // 128      # 2 d_in blocks
    BB = _CHUNK // 128    # 4 batch blocks per chunk
    W = UB * _CHUNK       # 2048

    b_rows = n_chunks * _CHUNK
    assert n_chunks % group == 0

    nc = bacc.Bacc("TRN2", target_bir_lowering=False, debug=False)

    x_d = nc.dram_tensor("x", [b_rows, _DIN], f32, kind="ExternalInput")
    h_d = nc.dram_tensor("h0", [b_rows, _UNITS], f32, kind="ExternalInput")
    K_d = nc.dram_tensor("Kw", [_DIN, _UNITS], f32, kind="ExternalInput")
    R_d = nc.dram_tensor("Rw", [_UNITS, _UNITS], f32, kind="ExternalInput")
    b_d = nc.dram_tensor("bv", [_UNITS], f32, kind="ExternalInput")
    s_d = nc.dram_tensor("sv", [_UNITS], f32, kind="ExternalInput")
    o_d = nc.dram_tensor("out", [b_rows, _UNITS], f32, kind="ExternalOutput")

    with tile.TileContext(nc) as tc, ExitStack() as ctx:
        wpool = ctx.enter_context(tc.tile_pool(name="w", bufs=1))
        stgpool = ctx.enter_context(tc.tile_pool(name="stg", bufs=2))
        iopool = ctx.enter_context(tc.tile_pool(name="io", bufs=2))
        hpool = ctx.enter_context(tc.tile_pool(name="h", bufs=group))
        xbpool = ctx.enter_context(tc.tile_pool(name="xb", bufs=group))
        vpool = ctx.enter_context(tc.tile_pool(name="vn", bufs=group + 2))
        v1pool = ctx.enter_context(tc.tile_pool(name="v1", bufs=group + 1))
        tpool = ctx.enter_context(tc.tile_pool(name="t", bufs=2))
        dpool = ctx.enter_context(tc.tile_pool(name="d", bufs=3))
        apool = ctx.enter_context(tc.tile_pool(name="acc", bufs=group))
        opool = ctx.enter_context(tc.tile_pool(name="o", bufs=2))
        pspool = ctx.enter_context(tc.tile_pool(name="ps", bufs=2, space="PSUM"))

        # ---- weights / constants ----
        R_sb = []
        for kb in range(UB):
            stg = stgpool.tile([128, _UNITS], f32, tag="stg")
            nc.sync.dma_start(out=stg[:], in_=R_d[kb * 128:(kb + 1) * 128, :])
            t_ = wpool.tile([128, _UNITS], bf16, tag=f"R{kb}")
            nc.vector.tensor_copy(t_[:], stg[:])
            R_sb.append(t_)
        K_sb = []
        for db in range(DB):
            stg = stgpool.tile([128, _UNITS], f32, tag="stg")
            nc.sync.dma_start(out=stg[:], in_=K_d[db * 128:(db + 1) * 128, :])
            t_ = wpool.tile([128, _UNITS], bf16, tag=f"K{db}")
            nc.vector.tensor_copy(t_[:], stg[:])
            K_sb.append(t_)
        bias_sb = wpool.tile([128, UB], f32, tag="bias")
        nc.sync.dma_start(out=bias_sb[:], in_=b_d[:].rearrange("(j p) -> p j", p=128))
        scale_sb = wpool.tile([128, UB], f32, tag="scale")
        nc.sync.dma_start(out=scale_sb[:], in_=s_d[:].rearrange("(j p) -> p j", p=128))
        ident = wpool.tile([128, 128], f32, tag="ident")
        make_identity(nc, ident[:])
        identW = wpool.tile([128, 128], bf16, tag="identW")
        nc.vector.tensor_copy(identW[:], ident[:])

        def mm(ps_ap, lhsT_ap, rhs_ap, start, stop):
            nc.tensor.matmul(ps_ap, lhsT_ap, rhs_ap, start=start, stop=stop)

        def emit_loads(chunks):
            loads = {}
            for c in chunks:
                r0 = c * _CHUNK
                xn, hn = [], []
                for bb in range(BB):
                    t_ = iopool.tile([128, _DIN], f32, tag=f"xn{bb}")
                    nc.sync.dma_start(
                        out=t_[:], in_=x_d[r0 + bb * 128:r0 + (bb + 1) * 128, :]
                    )
                    xn.append(t_)
                for bb in range(BB):
                    t_ = iopool.tile([128, _UNITS], f32, tag=f"hn{bb}")
                    nc.sync.dma_start(
                        out=t_[:], in_=h_d[r0 + bb * 128:r0 + (bb + 1) * 128, :]
                    )
                    hn.append(t_)
                loads[c] = (xn, hn)
            return loads

        groups = [list(range(g0, g0 + group))
                  for g0 in range(0, n_chunks, group)]
        next_loads = emit_loads(groups[0])

        for gi, chunks in enumerate(groups):
            st = {c: {} for c in chunks}
            loads = next_loads

            # ---- per-chunk init: transpose, xb ----
            for c in chunks:
                r0 = c * _CHUNK
                xn, hn = loads[c]

                xT = iopool.tile([128, DB * _CHUNK], bf16, tag="xT")
                ps = pspool.tile([128, W], f32, tag="ps")
                for db in range(DB):
                    for bb in range(BB):
                        nc.tensor.transpose(
                            ps[:, db * _CHUNK + bb * 128:db * _CHUNK + (bb + 1) * 128],
                            xn[bb][:, db * 128:(db + 1) * 128],
                            ident[:],
                        )
                nc.vector.tensor_copy(xT[:], ps[:, :DB * _CHUNK])

                hT = hpool.tile([128, W], f32, tag="hT")
                ps = pspool.tile([128, W], f32, tag="ps")
                for ub in range(UB):
                    for bb in range(BB):
                        nc.tensor.transpose(
                            ps[:, ub * _CHUNK + bb * 128:ub * _CHUNK + (bb + 1) * 128],
                            hn[bb][:, ub * 128:(ub + 1) * 128],
                            ident[:],
                        )
                nc.scalar.copy(hT[:], ps[:])
                st[c]["hT"] = hT

                # xbT = (x @ K).T (+ bias unless specialized away)
                xbT = xbpool.tile([128, W], bf16, tag="xbT")
                ps = pspool.tile([128, W], f32, tag="ps")
                for ub in range(UB):
                    for db in range(DB):
                        mm(
                            ps[:, ub * _CHUNK:(ub + 1) * _CHUNK],
                            K_sb[db][:, ub * 128:(ub + 1) * 128],
                            xT[:, db * _CHUNK:(db + 1) * _CHUNK],
                            start=(db == 0),
                            stop=(db == DB - 1),
                        )
                if specialize:
                    nc.vector.tensor_copy(xbT[:], ps[:])
                else:
                    for ub in range(UB):
                        nc.vector.tensor_scalar_add(
                            xbT[:, ub * _CHUNK:(ub + 1) * _CHUNK],
                            ps[:, ub * _CHUNK:(ub + 1) * _CHUNK],
                            bias_sb[:, ub:ub + 1],
                        )
                st[c]["xbT"] = xbT

                v1 = v1pool.tile([128, W], bf16, tag="v1")
                (nc.scalar.copy if cast_eng == "act" else nc.vector.tensor_copy)(
                    v1[:], hT[:]
                )
                st[c]["v1"] = v1
                st[c]["vcur"] = v1

            # ---- RK4 unfold steps, chunk-interleaved per stage ----
            for s_i in range(n_steps):
                for j in range(4):
                    cj = (_DT / 2.0) if j < 2 else _DT
                    # v1 casts are emitted one chunk late so they never
                    # head-of-line-block the next chunk's tanh on ACT (the
                    # cast waits on the DVE h-update; tanh frees PSUM)
                    pending_cast = []

                    def flush_cast():
                        cp = pending_cast.pop(0)
                        v1 = v1pool.tile([128, W], bf16, tag="v1")
                        (nc.scalar.copy if cast_eng == "act"
                         else nc.vector.tensor_copy)(v1[:], st[cp]["hT"][:])
                        st[cp]["v1"] = v1
                        st[cp]["vcur"] = v1

                    for c in chunks:
                        hT = st[c]["hT"]
                        vcur = st[c]["vcur"]
                        xbT = st[c]["xbT"]

                        if j == 3:
                            # E = d1 + 2*(d2 + d3), folded into d1's tile
                            # before this chunk's matmuls; the step-end
                            # critical chain is then tanh -> d4 -> E += d4
                            # -> one h stt
                            nc.vector.scalar_tensor_tensor(
                                st[c]["accA"][:], st[c]["accB"][:], 2.0,
                                st[c]["accA"][:], Alu.mult, Alu.add,
                            )

                        ps = pspool.tile([128, W], f32, tag="ps")
                        if not idmm:
                            nc.scalar.copy(ps[:], xbT[:])
                        for ub in range(UB):
                            sl = slice(ub * _CHUNK, (ub + 1) * _CHUNK)
                            if idmm:
                                mm(ps[:, sl], identW[:], xbT[:, sl],
                                   start=True, stop=False)
                            for kb in range(UB):
                                nc.tensor.matmul(
                                    ps[:, sl],
                                    R_sb[kb][:, ub * 128:(ub + 1) * 128],
                                    vcur[:, kb * _CHUNK:(kb + 1) * _CHUNK],
                                    start=False,
                                    stop=(kb == UB - 1),
                                    skip_group_check=(not idmm),
                                )

                        t_t = tpool.tile([128, W], bf16, tag="t")
                        nc.scalar.activation(t_t[:], ps[:], Act.Tanh)

                        if j == 0:
                            dtile = apool.tile([128, W], bf16, tag="accA")
                            st[c]["accA"] = dtile
                        elif j == 1:
                            dtile = apool.tile([128, W], bf16, tag="accB")
                            st[c]["accB"] = dtile
                        else:
                            dtile = dpool.tile([128, W], bf16, tag="d")

                        # d = s*t - vcur (scale pass vanishes when s == 1)
                        if specialize:
                            src = t_t
                        else:
                            src = tpool.tile([128, W], bf16, tag="ts")
                            for ub in range(UB):
                                sl = slice(ub * _CHUNK, (ub + 1) * _CHUNK)
                                if j % 2 == 0:
                                    nc.vector.tensor_scalar_mul(
                                        src[:, sl], t_t[:, sl],
                                        scale_sb[:, ub:ub + 1]
                                    )
                                else:
                                    nc.scalar.activation(
                                        src[:, sl], t_t[:, sl], Act.Identity,
                                        scale=scale_sb[:, ub:ub + 1],
                                    )
                        nc.vector.tensor_sub(dtile[:], src[:], vcur[:])

                        if j < 3:
                            # vn = v1 + cj*d via fast TS-mul + bf16 TT-add
                            dv = tpool.tile([128, W], bf16, tag="dv")
                            nc.vector.tensor_scalar_mul(dv[:], dtile[:], cj)
                            vn = vpool.tile([128, W], bf16, tag="vn")
                            nc.vector.tensor_add(vn[:], dv[:], st[c]["v1"][:])
                            st[c]["vcur"] = vn

                        if j == 2:
                            nc.vector.tensor_add(
                                st[c]["accB"][:], st[c]["accB"][:], dtile[:]
                            )
                        elif j == 3:
                            E = st[c]["accA"]
                            nc.vector.tensor_add(E[:], E[:], dtile[:])
                            nc.vector.scalar_tensor_tensor(
                                st[c]["hT"][:], E[:], _DT / 6.0,
                                st[c]["hT"][:], Alu.mult, Alu.add,
                            )
                            if s_i < n_steps - 1:
                                pending_cast.append(c)
                                if len(pending_cast) > 1:
                                    flush_cast()
                    while pending_cast:
                        flush_cast()

            # prefetch the next group's inputs before this group's output
            # phase so its transposes never wait on cold DMAs
            if gi + 1 < len(groups):
                next_loads = emit_loads(groups[gi + 1])

            # ---- output: transpose back and store ----
            for c in chunks:
                r0 = c * _CHUNK
                hT = st[c]["hT"]
                ps = pspool.tile([128, W], f32, tag="ps")
                for bb in range(BB):
                    for ub in range(UB):
                        nc.tensor.transpose(
                            ps[:, bb * _CHUNK + ub * 128:bb * _CHUNK + (ub + 1) * 128],
                            hT[:, ub * _CHUNK + bb * 128:ub * _CHUNK + (bb + 1) * 128],
                            ident[:],
                        )
                o_sb = opool.tile([128, W], f32, tag="o")
                nc.scalar.copy(o_sb[:], ps[:])
                for bb in range(BB):
                    nc.sync.dma_start(
                        out=o_d[r0 + bb * 128:r0 + (bb + 1) * 128, :],
                        in_=o_sb[:, bb * _CHUNK:(bb + 1) * _CHUNK],
                    )

    nc.compile()
    return nc


def _get_program(specialize):
    key = ("spec" if specialize else "gen")
    if key not in _cached:
        _cached[key] = _build_program(specialize=specialize)
    return _cached[key]


def _make_in_maps(inputs, hidden_state, kern, recurrent_kernel, bias, scale):
    def f(a):
        return np.ascontiguousarray(np.asarray(a), dtype=np.float32)

    x = f(inputs)
    h = f(hidden_state)
    shared = {
        "Kw": f(kern),
        "Rw": f(recurrent_kernel),
        "bv": f(bias),
        "sv": f(scale),
    }
    maps = []
    for c in range(_NCORES):
        sl = slice(c * _BLOCAL, (c + 1) * _BLOCAL)
        maps.append({"x": x[sl], "h0": h[sl], **shared})
    return maps


def _can_specialize(bias, scale):
    b = np.asarray(bias)
    s = np.asarray(scale)
    return bool(np.all(b == 0.0)) and bool(np.all(s == 1.0))


def _run(in_maps, specialize, trace=False):
    from concourse.bass_utils import run_bass_kernel_spmd

    nc = _get_program(specialize)
    res = run_bass_kernel_spmd(nc, in_maps, list(range(_NCORES)), trace=trace)
    out = np.concatenate(
        [res.results[i]["out"] for i in range(_NCORES)], axis=0
    ).astype(np.float32)
    return out, res


def kernel(inputs, hidden_state, kernel, recurrent_kernel, bias, scale):
    in_maps = _make_in_maps(inputs, hidden_state, kernel, recurrent_kernel,
                            bias, scale)
    # scale == 1 / bias == 0 (this problem's setup) takes the build that
    # skips those ops entirely; anything else takes the general build
    out, _ = _run(in_maps, specialize=_can_specialize(bias, scale))
    return out
